# revision 1
# baseline (speedup 1.0000x reference)
"""TRN2 Bass kernel for nn_Encoder_60112362275061 (GRU encoder).

B=128, T=1024, X=256, H=512 GRU; returns final hidden state h_T [B, H].
Data-parallel over 8 NeuronCores (16 batch rows per core); weights
replicated. See build_kernel() docstring for the per-core design.

Self-contained: hardcodes shapes/sharding; only imports the container
toolchain (concourse) and numpy.
"""

import sys

for _p in ("/opt/trn_rl_repo",):
    if _p not in sys.path:
        sys.path.insert(0, _p)

import numpy as np

import concourse.bass as bass
import concourse.mybir as mybir
from concourse.tile import TileContext

F32 = mybir.dt.float32
F16 = mybir.dt.float16

B, T_FULL, X, H = 128, 1024, 256, 512
NCORES = 8
BS = B // NCORES          # 16 batch rows per core
NG = 4                    # psum column groups == h chunks
HC = H // NG              # 128 h dims per chunk
GFD = 3 * HC              # 384 weight cols per group [r_j|z_j|n_j]
PB = 4 * HC               # 512 psum cols per step [hn|r|z|xn]
CH = 32                   # timesteps per For_i iteration
HB = CH // 2              # half-chunk (x staging granularity)
NR = 8                    # xg packed ring size (= steps per GEMM row tile)
SCD = 6                   # scatter issue distance (steps ahead)


def gate_perm():
    """Permutation P of the 3H gate dim: group j gets [r_j | z_j | n_j]."""
    idx = []
    for j in range(NG):
        idx.extend(range(j * HC, (j + 1) * HC))                  # r_j
        idx.extend(range(H + j * HC, H + (j + 1) * HC))          # z_j
        idx.extend(range(2 * H + j * HC, 2 * H + (j + 1) * HC))  # n_j
    return np.array(idx)


def host_prepare_weights(W_ih, W_hh, b_ih, b_hh):
    """Device weight tensors (shared by all cores).

    wpack [128, 2*1536 + 128 + 512] f16:
        wih k-chunk 0 | wih k-chunk 1 | I_mod | bias4 (rows 0:4)
    whhb [128, 4*3H + 128] f16: 4 contraction chunks of permuted W_hh^T
        plus the 128x128 identity for the PE transposes.

    I_mod is the seed-matmul stationary: identity plus, at row 32j+16
    (a junk batch slot), ones over columns [32j, 32j+16) -- so the bias
    row stored in the packed-xg junk partition lands on every real row
    of band j.
    """
    P = gate_perm()
    wih = np.ascontiguousarray(W_ih.T[:, P]).astype(np.float32)  # [256, 1536]
    whh = np.ascontiguousarray(W_hh.T[:, P]).astype(np.float32)  # [512, 1536]
    bih_p = b_ih[P].astype(np.float32)
    bhh_p = b_hh[P].astype(np.float32)
    comb = bih_p + bhh_p
    # psum layout per step: [hn | r | z | xn]
    bias4 = np.zeros((4, PB), np.float32)
    for j in range(NG):
        g = j * GFD
        bias4[j, 0:HC] = bhh_p[g + 2 * HC:g + 3 * HC]            # hn bias
        bias4[j, HC:3 * HC] = comb[g:g + 2 * HC]                 # r|z combined
        bias4[j, 3 * HC:4 * HC] = bih_p[g + 2 * HC:g + 3 * HC]   # xn bias
    imod = np.eye(128, dtype=np.float32)
    for j in range(NG):
        imod[32 * j + 16, 32 * j:32 * j + 16] = 1.0
    b4pad = np.zeros((128, PB), np.float32)
    b4pad[0:4] = bias4
    wpack = np.concatenate([wih[0:128], wih[128:256], imod, b4pad], axis=1)
    ident = np.eye(128, dtype=np.float32)
    # hh rhs col order per group: [n_j | r_j | z_j] to match psum layout
    hperm = np.concatenate([np.arange(j * GFD, (j + 1) * GFD)[
        np.r_[2 * HC:3 * HC, 0:2 * HC]] for j in range(NG)])
    whh = whh[:, hperm]
    whhb = np.concatenate(
        [whh[128 * c:128 * (c + 1)] for c in range(4)]
        + [ident], axis=1).astype(np.float16)
    return {"wpack": np.ascontiguousarray(wpack.astype(np.float16)),
            "whhb": np.ascontiguousarray(whhb)}


def host_prepare_x(x, core):
    """Per-core transposed x: [256, T*BS], col = t*BS + b."""
    xs = x[core * BS:(core + 1) * BS]                # [BS, T, X]
    t = xs.shape[1]
    return np.ascontiguousarray(
        xs.transpose(2, 1, 0).reshape(X, t * BS)).astype(np.float32)


def host_blob(x, wpack, core):
    """Per-core fp16 input blob: x halves then wpack."""
    xt = host_prepare_x(x, core).astype(np.float16)   # [256, T*BS]
    return np.ascontiguousarray(
        np.concatenate([xt[0:128], xt[128:256], wpack], axis=1))


def host_post(out_core):
    """[112, 128] packed h' -> [BS, H]."""
    out_core = np.asarray(out_core, dtype=np.float32)
    h = np.zeros((BS, H), np.float32)
    for j in range(NG):
        h[:, j * HC:(j + 1) * HC] = out_core[32 * j:32 * j + BS, :]
    return h


def build_kernel(T=T_FULL):
    """Per-core GRU program.

    Packed natural layout: batch rows at partitions 32j+b (h-chunk j,
    b<16); rows 32j+16..32j+32 are junk slots. Per step, one 2KB PSUM
    bank holds [hn|r|z|xn]: a single seed matmul (stationary I_mod,
    rhs = packed xg ring buffer, start=True) deposits x-side
    projections AND all biases (junk-row trick), then 4x4 fp16
    tile_position recurrent waves accumulate the h-side. The x-side
    GEMM runs at chunk granularity with M=128 (4 row tiles x 2 k-waves
    x 3 psum banks per 32 steps), its output scattered per-step into
    the packed ring by DMA.

    Tail: h' = n + z*(h - n). hT is produced by accumulating PE
    transposes of n and v=z*(h-n) so the transpose chain starts before
    h' exists; h' itself (DVE) runs off-chain in parallel.
    """
    assert T % CH == 0
    nc = bass.Bass("TRN2")

    WCOLS = 2 * 3 * H + 128 + PB
    xpack = nc.dram_tensor("xpack", [128, 2 * T * BS + WCOLS], F16,
                           kind="ExternalInput")
    whhb = nc.dram_tensor("whhb", [128, 4 * 3 * H + 128], F16,
                          kind="ExternalInput")
    hout = nc.dram_tensor("hout", [112, HC], F16, kind="ExternalOutput")

    sig = mybir.ActivationFunctionType.Sigmoid
    tanh = mybir.ActivationFunctionType.Tanh

    with TileContext(nc) as tc:
        with (
            tc.tile_pool(name="consts", bufs=1) as cpool,
            tc.tile_pool(name="state", bufs=1) as spool,
            tc.tile_pool(name="work", bufs=2) as wpool,
            tc.tile_pool(name="psumG", bufs=2, space="PSUM") as pgpool,
            tc.tile_pool(name="psumT", bufs=2, space="PSUM") as ptpool,
            tc.tile_pool(name="psumX", bufs=2, space="PSUM") as pxpool,
        ):
            # ---- resident constants ----
            wp_sb = cpool.tile([128, WCOLS], F16, tag="wpack")
            wh_sb = cpool.tile([128, 4 * 3 * H + 128], F16, tag="whhb")
            nc.sync.dma_start(out=wp_sb[:], in_=xpack[:, 2 * T * BS:])
            nc.sync.dma_start(out=wh_sb[:], in_=whhb[:, :])
            wih = [wp_sb[:, 0:3 * H], wp_sb[:, 3 * H:6 * H]]
            imod = wp_sb[:, 6 * H:6 * H + 128]
            b4_sb = wp_sb[0:4, 6 * H + 128:6 * H + 128 + PB]
            whh_k = [wh_sb[:, 3 * H * c:3 * H * (c + 1)] for c in range(4)]
            id_f16 = wh_sb[:, 12 * H:12 * H + 128]

            # ---- x staging (DRAM -> SBUF, half-chunk granularity) ----
            xcA = cpool.tile([128, 2, HB * BS], F16, tag="xcA")
            xcB = cpool.tile([128, 2, HB * BS], F16, tag="xcB")

            def dma_x(xc, off):
                nc.sync.dma_start(out=xc[:, 0, :],
                                  in_=xpack[:, bass.ds(off, HB * BS)])
                nc.sync.dma_start(
                    out=xc[:, 1, :],
                    in_=xpack[:, bass.ds(off + T * BS, HB * BS)])

            # ---- xg natural buffer (one chunk) + packed ring ----
            xgnat = cpool.tile([128, 4 * 1536], F16, tag="xgnat")
            # row tile r at cols r*1536
            xp = [cpool.tile([128, PB], F16, tag=f"xp{i}", name=f"xp{i}")
                  for i in range(NR)]
            for i in range(NR):
                nc.vector.memset(xp[i][:], 0.0)
                # bias rows live on junk partitions 32j+16 (DMA: DVE cannot
                # address non-32-aligned start partitions)
                nc.sync.dma_start(
                    out=xp[i].rearrange("(j bb) c -> j bb c", j=NG)[:, 16:17, :],
                    in_=b4_sb.rearrange("j (o c) -> j o c", o=1))

            # ---- persistent state (parity-indexed) ----
            hprev = [spool.tile([128, HC], F16, tag=f"hprev{p}",
                                name=f"hprev{p}") for p in range(2)]
            hT_sb = [spool.tile([128, 128], F16, tag=f"hT{p}",
                                name=f"hT{p}") for p in range(2)]
            nc.vector.memset(hprev[1][:], 0.0)
            nc.vector.memset(hT_sb[1][:], 0.0)

            gstate = {}

            def gemm_unit(r, n3, kw):
                """One x-GEMM matmul: row tile r, psum col block n3,
                contraction half kw. Returns a flush closure on kw==1."""
                xc = (xcA, xcB)[r // 2]
                lhsT = xc[:, kw, 128 * (r % 2):128 * (r % 2) + 128]
                if kw == 0:
                    gstate['px'] = pxpool.tile([128, 512], F32, tag="pX",
                                                name="pX")
                nc.tensor.matmul(gstate['px'][:, :], lhsT,
                                 wih[kw][:, 512 * n3:512 * (n3 + 1)],
                                 start=(kw == 0), stop=(kw == 1),
                                 skip_group_check=True)
                if kw == 1:
                    px = gstate['px']

                    def flush(r=r, n3=n3, px=px):
                        c0 = 1536 * r + 512 * n3
                        nc.vector.tensor_copy(xgnat[:, c0:c0 + 256],
                                              px[:, 0:256])
                        nc.vector.tensor_copy(xgnat[:, c0 + 256:c0 + 512],
                                              px[:, 256:512])
                    return flush
                return None

            def scatter(sp):
                """DMA xgnat row-tile -> packed ring buffer for step sp
                (step index within a chunk)."""
                r, tb = sp // NR, sp % NR
                for j in range(NG):
                    nc.sync.dma_start(
                        out=xp[sp % NR][32 * j:32 * j + BS, HC:PB],
                        in_=xgnat[16 * tb:16 * tb + 16,
                                  1536 * r + 384 * j:1536 * r + 384 * (j + 1)])

            def seed(s, pG):
                """Seed psum for step s: x projections + all biases."""
                nc.tensor.matmul(pG[:, :], imod, xp[s % NR][:, :],
                                 start=True, stop=False,
                                 tile_position=(0, 0),
                                 skip_group_check=True)

            # GEMM unit schedule: step -> list of (r, n3, kw)
            gsched = {}
            for r in range(4):
                base, stride = ((2, 1), (9, 1), (17, 1), (25, 1))[r]
                units = [(n3, kw) for n3 in range(3) for kw in range(2)]
                for i, (n3, kw) in enumerate(units):
                    gsched.setdefault(base + stride * i, []).append((r, n3, kw))

            def step(s, pG, pGnext):
                """Emit one timestep (recurrent waves + tail)."""
                p = s % 2

                # --- recurrent matmuls (fp16): 4 k-waves x 4 col groups ---
                for c in range(4):
                    for j in range(NG):
                        oo = slice(32 * j, 32 * (j + 1))
                        nc.tensor.matmul(
                            pG[oo, 0:GFD],
                            hT_sb[1 - p][:, 32 * c:32 * (c + 1)],
                            whh_k[c][:, j * GFD:(j + 1) * GFD],
                            start=False, stop=(c == 3 and j == NG - 1),
                            tile_position=(0, 32 * j),
                            skip_group_check=True)

                # --- seed next step's psum (runs during this tail) ---
                if pGnext is not None:
                    seed(s + 1, pGnext)

                # --- x-GEMM units assigned to this step slot ---
                flushes = []
                for unit in gsched.get(s, ()):
                    f = gemm_unit(*unit)
                    if f is not None:
                        flushes.append(f)

                # --- elementwise tail (fp16) ---
                rz = wpool.tile([128, 2 * HC], F16, tag="rz")
                m = wpool.tile([128, HC], F16, tag="m")
                a = wpool.tile([128, HC], F16, tag="a")
                n_t = wpool.tile([128, HC], F16, tag="n")
                u = wpool.tile([128, HC], F16, tag="u")
                v = wpool.tile([128, HC], F16, tag="v")

                nc.scalar.activation(rz[:], pG[:, HC:3 * HC], sig)
                nc.vector.tensor_tensor(m[:], rz[:, 0:HC], pG[:, 0:HC],
                                        mybir.AluOpType.mult)
                nc.vector.tensor_tensor(a[:], m[:], pG[:, 3 * HC:PB],
                                        mybir.AluOpType.add)
                nc.scalar.activation(n_t[:], a[:], tanh)
                # u = h - n ; v = z*u ; h' = n + v (h' off the hT chain)
                nc.vector.tensor_tensor(u[:], hprev[1 - p][:, :], n_t[:],
                                        mybir.AluOpType.subtract)
                nc.vector.tensor_tensor(v[:], rz[:, HC:2 * HC], u[:],
                                        mybir.AluOpType.mult)

                nc.vector.tensor_tensor(hprev[p][:, :], n_t[:], v[:],
                                        mybir.AluOpType.add)
                pT = ptpool.tile([128, 128], F16, tag="pT")
                nc.tensor.matmul(pT[:, :], hprev[p][:, :], id_f16,
                                 is_transpose=True, start=True, stop=True,
                                 skip_group_check=True)
                nc.vector.tensor_copy(hT_sb[p][:, :], pT[:, :])

                # pin flushes to the back half of this step's schedule so
                # the greedy scheduler cannot slot them into the tail chain
                if flushes:
                    with tc.tile_wait_until(0.0047 * s + 0.003):
                        for f in flushes:
                            f()

            # ================= prologue: chunk 0 =================
            dma_x(xcA, 0)
            dma_x(xcB, HB * BS)
            for r in range(4):
                for n3 in range(3):
                    fl = None
                    for kw in range(2):
                        f = gemm_unit(r, n3, kw)
                        fl = f or fl
                    fl()
            for sp in range(SCD):
                scatter(sp)

            # ================= main loop =================
            with tc.For_i(0, T * BS, CH * BS,
                          hint_engines=tuple(mybir.ALL_ENGINES)) as iv:
                dma_x(xcA, iv + CH * BS)       # next chunk, first half
                pG = pgpool.tile([128, PB], F32, tag="pG")
                seed(0, pG)
                for s in range(CH):
                    pGnext = (pgpool.tile([128, PB], F32, tag="pG",
                                          name="pG")
                              if s + 1 < CH else None)
                    step(s, pG, pGnext)
                    pG = pGnext
                    # scatter for step s+SCD (wraps into next chunk)
                    scatter((s + SCD) % CH)
                    if s == HB - 1:
                        dma_x(xcB, iv + CH * BS + HB * BS)

            # final h lives in hprev[(T-1) % 2]
            nc.sync.dma_start(out=hout[:, :], in_=hprev[(T - 1) % 2][0:112, :])

    _split_sync_waits(nc)
    return nc


def _split_sync_waits(nc):
    """Walrus codegen allows exactly ONE sync wait per instruction (the TPB
    events struct has a single wait slot). Tile emits multi-wait
    instructions (loop back-edge drains, barrier NoOps, cross-engine RAW
    joins); split the extras onto same-engine NoOps inserted immediately
    before -- the sequencer processes them in order, so semantics are
    identical."""
    for blk in nc.m.functions[0].blocks:
        i = 0
        while i < len(blk.instructions):
            inst = blk.instructions[i]
            si = getattr(inst, "sync_info", None)
            if si and si.on_wait and len(si.on_wait) > 1:
                waits = list(si.on_wait)
                si.on_wait = [waits[-1]]
                for w in waits[:-1]:
                    nop = mybir.InstNoOp(
                        name=nc.get_next_instruction_name(), ins=[], outs=[])
                    nop.engine = inst.engine
                    nop.sync_info = mybir.SyncInfo(on_wait=[w], on_update=[])
                    nc.register_instruction(nop)
                    blk.instructions.insert(i, nop)
                    i += 1
            i += 1


_NC_CACHE = {}


def run(x, W_ih, W_hh, b_ih, b_hh, trace=False):
    from concourse.bass_utils import run_bass_kernel_spmd

    x = np.asarray(x, dtype=np.float32)
    W_ih = np.asarray(W_ih, dtype=np.float32)
    W_hh = np.asarray(W_hh, dtype=np.float32)
    b_ih = np.asarray(b_ih, dtype=np.float32)
    b_hh = np.asarray(b_hh, dtype=np.float32)

    key = (x.shape[1],)
    if key not in _NC_CACHE:
        _NC_CACHE[key] = build_kernel(T=x.shape[1])
    nc = _NC_CACHE[key]

    wts = host_prepare_weights(W_ih, W_hh, b_ih, b_hh)
    in_maps = [{"xpack": host_blob(x, wts["wpack"], c), "whhb": wts["whhb"]}
               for c in range(NCORES)]
    res = run_bass_kernel_spmd(nc, in_maps, list(range(NCORES)), trace=trace)
    h = np.zeros((B, H), np.float32)
    for c in range(NCORES):
        h[c * BS:(c + 1) * BS] = host_post(np.asarray(res.results[c]["hout"]))
    return h, res


def kernel(x, W_ih, W_hh, b_ih, b_hh):
    h, _ = run(x, W_ih, W_hh, b_ih, b_hh)
    return h



# revision 7
# speedup vs baseline: 1.1549x; 1.1549x over previous
"""TRN2 Bass kernel for nn_Encoder_60112362275061 (GRU encoder).

B=128, T=1024, X=256, H=512 GRU; returns final hidden state h_T [B, H].
Data-parallel over 8 NeuronCores (16 batch rows per core); weights
replicated. See build_kernel() docstring for the per-core design.

Self-contained: hardcodes shapes/sharding; only imports the container
toolchain (concourse) and numpy.
"""

import sys

for _p in ("/opt/trn_rl_repo",):
    if _p not in sys.path:
        sys.path.insert(0, _p)

import numpy as np

import concourse.bass as bass
import concourse.mybir as mybir
from concourse.tile import TileContext

F32 = mybir.dt.float32
F16 = mybir.dt.float16

B, T_FULL, X, H = 128, 1024, 256, 512
NCORES = 8
BS = B // NCORES          # 16 batch rows per core
NG = 4                    # psum column groups == h chunks
HC = H // NG              # 128 h dims per chunk
GFD = 3 * HC              # 384 weight cols per group [r_j|z_j|n_j]
PB = 4 * HC               # 512 xp cols per step [r|z|xn|hn-bias]
CH = 32                   # timesteps per For_i iteration
HB = CH // 2              # half-chunk (x staging granularity)
NR = 8                    # xg packed ring size (= steps per GEMM row tile)
SCD = 6                   # scatter issue distance (steps ahead)
FLUSH_MS = 0.0032         # x-flush schedule pin slope (ms per step)
JUNK_OFF = False          # debug: disable HAM warmer matmuls
SINGLE_T = True           # debug: single transpose of h' (baseline-style)


def gate_perm():
    """Permutation P of the 3H gate dim: group j gets [r_j | z_j | n_j]."""
    idx = []
    for j in range(NG):
        idx.extend(range(j * HC, (j + 1) * HC))                  # r_j
        idx.extend(range(H + j * HC, H + (j + 1) * HC))          # z_j
        idx.extend(range(2 * H + j * HC, 2 * H + (j + 1) * HC))  # n_j
    return np.array(idx)


def host_prepare_weights(W_ih, W_hh, b_ih, b_hh):
    """Device weight tensors (shared by all cores).

    wpack [128, 2*1536 + 128 + 512] f16:
        wih k-chunk 0 | wih k-chunk 1 | I_mod | bias4 (rows 0:4)
    whhb [128, 4*3H + 128] f16: 4 contraction chunks of permuted W_hh^T
        (group j cols ordered [r_j|z_j|n_j]) plus the 128x128 identity
        for the PE transposes.

    I_mod is the seed-matmul stationary: identity plus, at row 32j+16
    (a junk batch slot), ones over columns [32j, 32j+16) -- so the bias
    row stored in the packed-xg junk partition lands on every real row
    of band j.
    """
    P = gate_perm()
    wih = np.ascontiguousarray(W_ih.T[:, P]).astype(np.float32)  # [256, 1536]
    whh = np.ascontiguousarray(W_hh.T[:, P]).astype(np.float32)  # [512, 1536]
    bih_p = b_ih[P].astype(np.float32)
    bhh_p = b_hh[P].astype(np.float32)
    comb = bih_p + bhh_p
    # xp/bias layout per step: [r | z | xn | hn]
    bias4 = np.zeros((4, PB), np.float32)
    for j in range(NG):
        g = j * GFD
        bias4[j, 0:2 * HC] = comb[g:g + 2 * HC]                  # r|z combined
        bias4[j, 2 * HC:3 * HC] = bih_p[g + 2 * HC:g + 3 * HC]   # xn bias
        bias4[j, 3 * HC:4 * HC] = bhh_p[g + 2 * HC:g + 3 * HC]   # hn bias
    imod = np.eye(128, dtype=np.float32)
    for j in range(NG):
        imod[32 * j + 16, 32 * j:32 * j + 16] = 1.0
    b4pad = np.zeros((128, PB), np.float32)
    b4pad[0:4] = bias4
    wpack = np.concatenate([wih[0:128], wih[128:256], imod, b4pad], axis=1)
    ident = np.eye(128, dtype=np.float32)
    whhb = np.concatenate(
        [whh[128 * c:128 * (c + 1)] for c in range(4)]
        + [ident], axis=1).astype(np.float16)
    return {"wpack": np.ascontiguousarray(wpack.astype(np.float16)),
            "whhb": np.ascontiguousarray(whhb)}


def host_prepare_x(x, core):
    """Per-core transposed x: [256, T*BS], col = t*BS + b."""
    xs = x[core * BS:(core + 1) * BS]                # [BS, T, X]
    t = xs.shape[1]
    return np.ascontiguousarray(
        xs.transpose(2, 1, 0).reshape(X, t * BS)).astype(np.float32)


def host_blob(x, wpack, core):
    """Per-core fp16 input blob: x halves then wpack."""
    xt = host_prepare_x(x, core).astype(np.float16)   # [256, T*BS]
    return np.ascontiguousarray(
        np.concatenate([xt[0:128], xt[128:256], wpack], axis=1))


def host_post(out_core):
    """[112, 128] packed h' -> [BS, H]."""
    out_core = np.asarray(out_core, dtype=np.float32)
    h = np.zeros((BS, H), np.float32)
    for j in range(NG):
        h[:, j * HC:(j + 1) * HC] = out_core[32 * j:32 * j + BS, :]
    return h


def build_kernel(T=T_FULL):
    """Per-core GRU program.

    Packed natural layout: batch rows at partitions 32j+b (h-chunk j,
    b<16); rows 32j+16..32j+32 are junk slots. Per step, psum bank A
    [128,384] holds [r|z|xn] and bank B [128,128] holds [hn]: seed
    matmuls (stationary I_mod, rhs = packed xg ring, start=True)
    deposit x-side projections AND all biases (junk-row trick), then
    4x4 fp16 column-band waves accumulate the h side. A-waves (r|z,
    N=256) are emitted before B-waves (hn, N=128) so the r sigmoid's
    psum dependency clears early. The x-side GEMM runs at chunk
    granularity with M=128, scattered per-step into the packed ring.

    Tail chain: sig(r) -> m=r*hn -> a=m+xn -> tanh -> u=h-n -> v=z*u
    -> T(v) -> copy(hT). sig(z), h'=n+v, and T(n) run off-chain. The
    hT for the next step's waves accumulates T(n)+T(v) in PSUM.

    Junk matmuls (anchored on tail intermediates) keep TensorE busy
    through the tail so the HAM clock gate stays at full rate.
    """
    assert T % CH == 0
    nc = bass.Bass("TRN2")

    WCOLS = 2 * 3 * H + 128 + PB
    xpack = nc.dram_tensor("xpack", [128, 2 * T * BS + WCOLS], F16,
                           kind="ExternalInput")
    whhb = nc.dram_tensor("whhb", [128, 4 * 3 * H + 128], F16,
                          kind="ExternalInput")
    hout = nc.dram_tensor("hout", [112, HC], F16, kind="ExternalOutput")

    sig = mybir.ActivationFunctionType.Sigmoid
    tanh = mybir.ActivationFunctionType.Tanh

    with TileContext(nc) as tc:
        with (
            tc.tile_pool(name="consts", bufs=1) as cpool,
            tc.tile_pool(name="state", bufs=1) as spool,
            tc.tile_pool(name="work", bufs=2) as wpool,
            tc.tile_pool(name="psumA", bufs=2, space="PSUM") as papool,
            tc.tile_pool(name="psumB", bufs=2, space="PSUM") as pbpool,
            tc.tile_pool(name="psumT", bufs=1, space="PSUM") as ptpool,
            tc.tile_pool(name="psumX", bufs=2, space="PSUM") as pxpool,
            tc.tile_pool(name="psumJ", bufs=1, space="PSUM") as pjpool,
        ):
            # ---- resident constants ----
            wp_sb = cpool.tile([128, WCOLS], F16, tag="wpack")
            wh_sb = cpool.tile([128, 4 * 3 * H + 128], F16, tag="whhb")
            nc.sync.dma_start(out=wp_sb[:], in_=xpack[:, 2 * T * BS:])
            nc.sync.dma_start(out=wh_sb[:], in_=whhb[:, :])
            wih = [wp_sb[:, 0:3 * H], wp_sb[:, 3 * H:6 * H]]
            imod = wp_sb[:, 6 * H:6 * H + 128]
            b4_sb = wp_sb[0:4, 6 * H + 128:6 * H + 128 + PB]
            whh_k = [wh_sb[:, 3 * H * c:3 * H * (c + 1)] for c in range(4)]
            id_f16 = wh_sb[:, 12 * H:12 * H + 128]

            # ---- x staging (DRAM -> SBUF, half-chunk granularity) ----
            xcA = cpool.tile([128, 2, HB * BS], F16, tag="xcA")
            xcB = cpool.tile([128, 2, HB * BS], F16, tag="xcB")

            def dma_x(xc, off):
                nc.sync.dma_start(out=xc[:, 0, :],
                                  in_=xpack[:, bass.ds(off, HB * BS)])
                nc.sync.dma_start(
                    out=xc[:, 1, :],
                    in_=xpack[:, bass.ds(off + T * BS, HB * BS)])

            # ---- xg natural buffer (one chunk) + packed ring ----
            xgnat = cpool.tile([128, 4 * 1536], F16, tag="xgnat")
            xp = [cpool.tile([128, PB], F16, tag=f"xp{i}", name=f"xp{i}")
                  for i in range(NR)]
            for i in range(NR):
                nc.vector.memset(xp[i][:], 0.0)
                # bias rows live on junk partitions 32j+16 (DMA: DVE cannot
                # address non-32-aligned start partitions)
                nc.sync.dma_start(
                    out=xp[i].rearrange("(j bb) c -> j bb c", j=NG)[:, 16:17, :],
                    in_=b4_sb.rearrange("j (o c) -> j o c", o=1))

            # ---- persistent state (parity-indexed) ----
            hprev = [spool.tile([128, HC], F16, tag=f"hprev{p}",
                                name=f"hprev{p}") for p in range(2)]
            hT_sb = [spool.tile([128, 128], F16, tag=f"hT{p}",
                                name=f"hT{p}") for p in range(2)]
            nc.vector.memset(hprev[1][:], 0.0)
            nc.vector.memset(hT_sb[1][:], 0.0)

            # ---- junk psum target for HAM warmers ----
            junk_ps = pjpool.tile([128, 256], F32, tag="junk")

            def junkmm(k, anchor):
                """k matmuls with no consumers: keep the PE busy (and the
                HAM clock gate warm) through the tail. `anchor` (a tail
                intermediate) orders them into the right idle window."""
                if JUNK_OFF:
                    return
                for _ in range(k):
                    nc.tensor.matmul(junk_ps[:, :], anchor[:, 0:128],
                                     wh_sb[:, 0:256], start=True, stop=True,
                                     skip_group_check=True)

            gstate = {}

            def gemm_unit(r, n3, kw):
                """One x-GEMM matmul: row tile r, psum col block n3,
                contraction half kw. Returns a flush closure on kw==1."""
                xc = (xcA, xcB)[r // 2]
                lhsT = xc[:, kw, 128 * (r % 2):128 * (r % 2) + 128]
                if kw == 0:
                    gstate['px'] = pxpool.tile([128, 512], F32, tag="pX",
                                                name="pX")
                nc.tensor.matmul(gstate['px'][:, :], lhsT,
                                 wih[kw][:, 512 * n3:512 * (n3 + 1)],
                                 start=(kw == 0), stop=(kw == 1),
                                 skip_group_check=True)
                if kw == 1:
                    px = gstate['px']

                    def flush(r=r, n3=n3, px=px):
                        c0 = 1536 * r + 512 * n3
                        nc.vector.tensor_copy(xgnat[:, c0:c0 + 256],
                                              px[:, 0:256])
                        nc.vector.tensor_copy(xgnat[:, c0 + 256:c0 + 512],
                                              px[:, 256:512])
                    return flush
                return None

            def scatter(sp):
                """DMA xgnat row-tile -> packed ring buffer for step sp
                (step index within a chunk). xgnat group layout [r|z|n]
                lands on xp cols [r|z|xn] = 0:384."""
                r, tb = sp // NR, sp % NR
                for j in range(NG):
                    nc.sync.dma_start(
                        out=xp[sp % NR][32 * j:32 * j + BS, 0:GFD],
                        in_=xgnat[16 * tb:16 * tb + 16,
                                  1536 * r + 384 * j:1536 * r + 384 * (j + 1)])

            def seed(s, A, Bp):
                """Seed psum for step s: x projections + all biases."""
                nc.tensor.matmul(A[:, :], imod, xp[s % NR][:, 0:GFD],
                                 start=True, stop=False,
                                 tile_position=(0, 0),
                                 skip_group_check=True)
                nc.tensor.matmul(Bp[:, :], imod, xp[s % NR][:, GFD:PB],
                                 start=True, stop=False,
                                 tile_position=(0, 0),
                                 skip_group_check=True)

            # GEMM unit schedule: step -> list of (r, n3, kw)
            gsched = {}
            for r in range(4):
                base, stride = ((2, 1), (9, 1), (17, 1), (25, 1))[r]
                units = [(n3, kw) for n3 in range(3) for kw in range(2)]
                for i, (n3, kw) in enumerate(units):
                    gsched.setdefault(base + stride * i, []).append((r, n3, kw))

            def step(s, A, Bp, ABnext):
                """Emit one timestep (recurrent waves + tail)."""
                p = s % 2

                # --- A-waves (r|z, N=256) then B-waves (hn, N=128) ---
                for c in range(4):
                    for j in range(NG):
                        oo = slice(32 * j, 32 * (j + 1))
                        nc.tensor.matmul(
                            A[oo, 0:2 * HC],
                            hT_sb[1 - p][:, 32 * c:32 * (c + 1)],
                            whh_k[c][:, j * GFD:j * GFD + 2 * HC],
                            start=False, stop=(c == 3 and j == NG - 1),
                            tile_position=(0, 32 * j),
                            skip_group_check=True)
                for c in range(4):
                    for j in range(NG):
                        oo = slice(32 * j, 32 * (j + 1))
                        nc.tensor.matmul(
                            Bp[oo, 0:HC],
                            hT_sb[1 - p][:, 32 * c:32 * (c + 1)],
                            whh_k[c][:, j * GFD + 2 * HC:(j + 1) * GFD],
                            start=False, stop=(c == 3 and j == NG - 1),
                            tile_position=(0, 32 * j),
                            skip_group_check=True)

                # --- seed next step's psum (runs during this tail) ---
                if ABnext is not None:
                    seed(s + 1, *ABnext)

                # --- x-GEMM units assigned to this step slot ---
                flushes = []
                for unit in gsched.get(s, ()):
                    f = gemm_unit(*unit)
                    if f is not None:
                        flushes.append(f)

                # --- elementwise tail (fp16) ---
                r_t = wpool.tile([128, HC], F16, tag="r")
                z_t = wpool.tile([128, HC], F16, tag="z")
                m = wpool.tile([128, HC], F16, tag="m")
                a = wpool.tile([128, HC], F16, tag="a")
                n_t = wpool.tile([128, HC], F16, tag="n")
                u = wpool.tile([128, HC], F16, tag="u")
                v = wpool.tile([128, HC], F16, tag="v")
                pT = ptpool.tile([128, 128], F16, tag="pT")

                nc.scalar.activation(r_t[:], A[:, 0:HC], sig)
                nc.scalar.activation(z_t[:], A[:, HC:2 * HC], sig)
                nc.vector.tensor_tensor(m[:], r_t[:], Bp[:, 0:HC],
                                        mybir.AluOpType.mult)
                junkmm(3, r_t)
                nc.vector.tensor_tensor(a[:], m[:], A[:, 2 * HC:3 * HC],
                                        mybir.AluOpType.add)
                nc.scalar.activation(n_t[:], a[:], tanh)
                nc.vector.tensor_tensor(u[:], hprev[1 - p][:, :], n_t[:],
                                        mybir.AluOpType.subtract)
                if not SINGLE_T:
                    # T(n) accumulates into pT; T(v) completes h'^T
                    nc.tensor.matmul(pT[:, :], n_t[:, :], id_f16,
                                     is_transpose=True, start=True, stop=False,
                                     skip_group_check=True)
                nc.vector.tensor_tensor(v[:], z_t[:], u[:],
                                        mybir.AluOpType.mult)
                junkmm(2, u)
                if not SINGLE_T:
                    nc.tensor.matmul(pT[:, :], v[:, :], id_f16,
                                     is_transpose=True, start=False, stop=True,
                                     skip_group_check=True)
                nc.vector.tensor_tensor(hprev[p][:, :], n_t[:], v[:],
                                        mybir.AluOpType.add)
                if SINGLE_T:
                    nc.tensor.matmul(pT[:, :], hprev[p][:, :], id_f16,
                                     is_transpose=True, start=True, stop=True,
                                     skip_group_check=True)
                junkmm(2, hprev[p])
                nc.vector.tensor_copy(hT_sb[p][:, :], pT[:, :])

                # pin flushes to the back half of this step's schedule so
                # the greedy scheduler cannot slot them into the tail chain
                if flushes:
                    with tc.tile_wait_until(FLUSH_MS * s + 0.003):
                        for f in flushes:
                            f()

            # ================= prologue: chunk 0 =================
            dma_x(xcA, 0)
            dma_x(xcB, HB * BS)
            for r in range(4):
                for n3 in range(3):
                    fl = None
                    for kw in range(2):
                        f = gemm_unit(r, n3, kw)
                        fl = f or fl
                    fl()
            for sp in range(SCD):
                scatter(sp)

            # ================= main loop =================
            with tc.For_i(0, T * BS, CH * BS,
                          hint_engines=tuple(mybir.ALL_ENGINES)) as iv:
                dma_x(xcA, iv + CH * BS)       # next chunk, first half
                A = papool.tile([128, GFD], F32, tag="pA")
                Bp = pbpool.tile([128, HC], F32, tag="pB")
                seed(0, A, Bp)
                for s in range(CH):
                    if s + 1 < CH:
                        ABnext = (papool.tile([128, GFD], F32, tag="pA",
                                              name="pA"),
                                  pbpool.tile([128, HC], F32, tag="pB",
                                              name="pB"))
                    else:
                        ABnext = None
                    step(s, A, Bp, ABnext)
                    if ABnext is not None:
                        A, Bp = ABnext
                    # scatter for step s+SCD (wraps into next chunk)
                    scatter((s + SCD) % CH)
                    if s == HB - 1:
                        dma_x(xcB, iv + CH * BS + HB * BS)

            # final h lives in hprev[(T-1) % 2]
            nc.sync.dma_start(out=hout[:, :], in_=hprev[(T - 1) % 2][0:112, :])

    _split_sync_waits(nc)
    return nc


def _split_sync_waits(nc):
    """Walrus codegen allows exactly ONE sync wait per instruction (the TPB
    events struct has a single wait slot). Tile emits multi-wait
    instructions (loop back-edge drains, barrier NoOps, cross-engine RAW
    joins); split the extras onto same-engine NoOps inserted immediately
    before -- the sequencer processes them in order, so semantics are
    identical."""
    for blk in nc.m.functions[0].blocks:
        i = 0
        while i < len(blk.instructions):
            inst = blk.instructions[i]
            si = getattr(inst, "sync_info", None)
            if si and si.on_wait and len(si.on_wait) > 1:
                waits = list(si.on_wait)
                si.on_wait = [waits[-1]]
                for w in waits[:-1]:
                    nop = mybir.InstNoOp(
                        name=nc.get_next_instruction_name(), ins=[], outs=[])
                    nop.engine = inst.engine
                    nop.sync_info = mybir.SyncInfo(on_wait=[w], on_update=[])
                    nc.register_instruction(nop)
                    blk.instructions.insert(i, nop)
                    i += 1
            i += 1


_NC_CACHE = {}


def run(x, W_ih, W_hh, b_ih, b_hh, trace=False):
    from concourse.bass_utils import run_bass_kernel_spmd

    x = np.asarray(x, dtype=np.float32)
    W_ih = np.asarray(W_ih, dtype=np.float32)
    W_hh = np.asarray(W_hh, dtype=np.float32)
    b_ih = np.asarray(b_ih, dtype=np.float32)
    b_hh = np.asarray(b_hh, dtype=np.float32)

    key = (x.shape[1],)
    if key not in _NC_CACHE:
        _NC_CACHE[key] = build_kernel(T=x.shape[1])
    nc = _NC_CACHE[key]

    wts = host_prepare_weights(W_ih, W_hh, b_ih, b_hh)
    in_maps = [{"xpack": host_blob(x, wts["wpack"], c), "whhb": wts["whhb"]}
               for c in range(NCORES)]
    res = run_bass_kernel_spmd(nc, in_maps, list(range(NCORES)), trace=trace)
    h = np.zeros((B, H), np.float32)
    for c in range(NCORES):
        h[c * BS:(c + 1) * BS] = host_post(np.asarray(res.results[c]["hout"]))
    return h, res


def kernel(x, W_ih, W_hh, b_ih, b_hh):
    h, _ = run(x, W_ih, W_hh, b_ih, b_hh)
    return h


# revision 14
# speedup vs baseline: 1.2113x; 1.0489x over previous
"""TRN2 Bass kernel for nn_Encoder_60112362275061 (GRU encoder).

B=128, T=1024, X=256, H=512 GRU; returns final hidden state h_T [B, H].
Data-parallel over 8 NeuronCores (16 batch rows per core); weights
replicated. See build_kernel() docstring for the per-core design.

Self-contained: hardcodes shapes/sharding; only imports the container
toolchain (concourse) and numpy.
"""

import sys

for _p in ("/opt/trn_rl_repo",):
    if _p not in sys.path:
        sys.path.insert(0, _p)

import numpy as np

import concourse.bass as bass
import concourse.mybir as mybir
from concourse.tile import TileContext

F32 = mybir.dt.float32
F16 = mybir.dt.float16

B, T_FULL, X, H = 128, 1024, 256, 512
NCORES = 8
BS = B // NCORES          # 16 batch rows per core
NG = 4                    # psum column groups == h chunks
HC = H // NG              # 128 h dims per chunk
GFD = 3 * HC              # 384 weight cols per group [r_j|z_j|n_j]
PB = 4 * HC               # 512 xp cols per step [r|z|xn|hn-bias]
CH = 64                   # timesteps per For_i iteration
HB = CH // 2              # half-chunk (x staging granularity)
NR = 8                    # xg packed ring size (= steps per GEMM row tile)
SCD = 6                   # scatter issue distance (steps ahead)
FLUSH_MS = 0.0032         # x-flush schedule pin slope (ms per step)
JUNK_OFF = False          # debug: disable HAM warmer matmuls
SINGLE_T = False          # single transpose of h' vs fp32 T(n)+T(v) accumulate
NRT = CH * 16 // 128      # x-GEMM row tiles per chunk


def gate_perm():
    """Permutation P of the 3H gate dim: group j gets [r_j | z_j | n_j]."""
    idx = []
    for j in range(NG):
        idx.extend(range(j * HC, (j + 1) * HC))                  # r_j
        idx.extend(range(H + j * HC, H + (j + 1) * HC))          # z_j
        idx.extend(range(2 * H + j * HC, 2 * H + (j + 1) * HC))  # n_j
    return np.array(idx)


def host_prepare_weights(W_ih, W_hh, b_ih, b_hh):
    """Device weight tensors (shared by all cores).

    wpack [128, 2*1536 + 128 + 512] f16:
        wih k-chunk 0 | wih k-chunk 1 | I_mod | bias4 (rows 0:4)
    whhb [128, 4*3H + 128] f16: 4 contraction chunks of permuted W_hh^T
        (group j cols ordered [r_j|z_j|n_j]) plus the 128x128 identity
        for the PE transposes.

    I_mod is the seed-matmul stationary: identity plus, at row 32j+16
    (a junk batch slot), ones over columns [32j, 32j+16) -- so the bias
    row stored in the packed-xg junk partition lands on every real row
    of band j.
    """
    P = gate_perm()
    wih = np.ascontiguousarray(W_ih.T[:, P]).astype(np.float32)  # [256, 1536]
    whh = np.ascontiguousarray(W_hh.T[:, P]).astype(np.float32)  # [512, 1536]
    bih_p = b_ih[P].astype(np.float32)
    bhh_p = b_hh[P].astype(np.float32)
    comb = bih_p + bhh_p
    # xp/bias layout per step: [r | z | xn | hn]
    bias4 = np.zeros((4, PB), np.float32)
    for j in range(NG):
        g = j * GFD
        bias4[j, 0:2 * HC] = comb[g:g + 2 * HC]                  # r|z combined
        bias4[j, 2 * HC:3 * HC] = bih_p[g + 2 * HC:g + 3 * HC]   # xn bias
        bias4[j, 3 * HC:4 * HC] = bhh_p[g + 2 * HC:g + 3 * HC]   # hn bias
    imod = np.eye(128, dtype=np.float32)
    for j in range(NG):
        imod[32 * j + 16, 32 * j:32 * j + 16] = 1.0
    b4pad = np.zeros((128, PB), np.float32)
    b4pad[0:4] = bias4
    wpack = np.concatenate([wih[0:128], wih[128:256], imod, b4pad], axis=1)
    ident = np.eye(128, dtype=np.float32)
    whhb = np.concatenate(
        [whh[128 * c:128 * (c + 1)] for c in range(4)]
        + [ident], axis=1).astype(np.float16)
    return {"wpack": np.ascontiguousarray(wpack.astype(np.float16)),
            "whhb": np.ascontiguousarray(whhb)}


def host_prepare_x(x, core):
    """Per-core transposed x: [256, T*BS], col = t*BS + b."""
    xs = x[core * BS:(core + 1) * BS]                # [BS, T, X]
    t = xs.shape[1]
    return np.ascontiguousarray(
        xs.transpose(2, 1, 0).reshape(X, t * BS)).astype(np.float32)


def host_blob(x, wpack, core):
    """Per-core fp16 input blob: x halves then wpack."""
    xt = host_prepare_x(x, core).astype(np.float16)   # [256, T*BS]
    return np.ascontiguousarray(
        np.concatenate([xt[0:128], xt[128:256], wpack], axis=1))


def host_post(out_core):
    """[112, 128] packed h' -> [BS, H]."""
    out_core = np.asarray(out_core, dtype=np.float32)
    h = np.zeros((BS, H), np.float32)
    for j in range(NG):
        h[:, j * HC:(j + 1) * HC] = out_core[32 * j:32 * j + BS, :]
    return h


def build_kernel(T=T_FULL):
    """Per-core GRU program.

    Packed natural layout: batch rows at partitions 32j+b (h-chunk j,
    b<16); rows 32j+16..32j+32 are junk slots. Per step, psum bank A
    [128,384] holds [r|z|xn] and bank B [128,128] holds [hn]: seed
    matmuls (stationary I_mod, rhs = packed xg ring, start=True)
    deposit x-side projections AND all biases (junk-row trick), then
    4x4 fp16 column-band waves accumulate the h side. A-waves (r|z,
    N=256) are emitted before B-waves (hn, N=128) so the r sigmoid's
    psum dependency clears early. The x-side GEMM runs at chunk
    granularity with M=128, scattered per-step into the packed ring.

    Tail chain: sig(r) -> m=r*hn -> a=m+xn -> tanh -> u=h-n -> v=z*u
    -> T(v) -> copy(hT). sig(z), h'=n+v, and T(n) run off-chain. The
    hT for the next step's waves accumulates T(n)+T(v) in PSUM.

    Junk matmuls (anchored on tail intermediates) keep TensorE busy
    through the tail so the HAM clock gate stays at full rate.
    """
    assert T % CH == 0
    nc = bass.Bass("TRN2")

    WCOLS = 2 * 3 * H + 128 + PB
    xpack = nc.dram_tensor("xpack", [128, 2 * T * BS + WCOLS], F16,
                           kind="ExternalInput")
    whhb = nc.dram_tensor("whhb", [128, 4 * 3 * H + 128], F16,
                          kind="ExternalInput")
    auxw = nc.dram_tensor("auxw", [128, 128], F32, kind="ExternalInput")
    hout = nc.dram_tensor("hout", [112, HC], F16, kind="ExternalOutput")

    sig = mybir.ActivationFunctionType.Sigmoid
    tanh = mybir.ActivationFunctionType.Tanh

    with TileContext(nc) as tc:
        with (
            tc.tile_pool(name="consts", bufs=1) as cpool,
            tc.tile_pool(name="state", bufs=1) as spool,
            tc.tile_pool(name="work", bufs=2) as wpool,
            tc.tile_pool(name="psumA", bufs=2, space="PSUM") as papool,
            tc.tile_pool(name="psumB", bufs=2, space="PSUM") as pbpool,
            tc.tile_pool(name="psumT", bufs=1, space="PSUM") as ptpool,
            tc.tile_pool(name="psumX", bufs=2, space="PSUM") as pxpool,
            tc.tile_pool(name="psumJ", bufs=1, space="PSUM") as pjpool,
        ):
            # ---- resident constants ----
            wp_sb = cpool.tile([128, WCOLS], F16, tag="wpack")
            wh_sb = cpool.tile([128, 4 * 3 * H + 128], F16, tag="whhb")
            nc.sync.dma_start(out=wp_sb[:], in_=xpack[:, 2 * T * BS:])
            nc.sync.dma_start(out=wh_sb[:], in_=whhb[:, :])
            id32_sb = cpool.tile([128, 128], F32, tag="id32")
            nc.sync.dma_start(out=id32_sb[:], in_=auxw[:, :])
            id_f32 = id32_sb[:, :]
            wih = [wp_sb[:, 0:3 * H], wp_sb[:, 3 * H:6 * H]]
            imod = wp_sb[:, 6 * H:6 * H + 128]
            b4_sb = wp_sb[0:4, 6 * H + 128:6 * H + 128 + PB]
            whh_k = [wh_sb[:, 3 * H * c:3 * H * (c + 1)] for c in range(4)]
            id_f16 = wh_sb[:, 12 * H:12 * H + 128]

            # ---- x staging (DRAM -> SBUF, half-chunk granularity) ----
            xcA = cpool.tile([128, 2, HB * BS], F16, tag="xcA")
            xcB = cpool.tile([128, 2, HB * BS], F16, tag="xcB")

            def dma_x(xc, off):
                nc.sync.dma_start(out=xc[:, 0, :],
                                  in_=xpack[:, bass.ds(off, HB * BS)])
                nc.sync.dma_start(
                    out=xc[:, 1, :],
                    in_=xpack[:, bass.ds(off + T * BS, HB * BS)])

            # ---- xg natural buffer (one chunk) + packed ring ----
            xgnat = cpool.tile([128, NRT * 1536], F16, tag="xgnat")
            xp = [cpool.tile([128, PB], F16, tag=f"xp{i}", name=f"xp{i}")
                  for i in range(NR)]
            for i in range(NR):
                nc.vector.memset(xp[i][:], 0.0)
                # bias rows live on junk partitions 32j+16 (DMA: DVE cannot
                # address non-32-aligned start partitions)
                nc.sync.dma_start(
                    out=xp[i].rearrange("(j bb) c -> j bb c", j=NG)[:, 16:17, :],
                    in_=b4_sb.rearrange("j (o c) -> j o c", o=1))

            # ---- persistent state (parity-indexed) ----
            hprev = [spool.tile([128, HC], F16, tag=f"hprev{p}",
                                name=f"hprev{p}") for p in range(2)]
            hT_sb = [spool.tile([128, 128], F16, tag=f"hT{p}",
                                name=f"hT{p}") for p in range(2)]
            nc.vector.memset(hprev[1][:], 0.0)
            nc.vector.memset(hT_sb[1][:], 0.0)

            # ---- junk psum target for HAM warmers ----
            junk_ps = pjpool.tile([128, 256], F32, tag="junk")

            def junkmm(k, anchor):
                """k matmuls with no consumers: keep the PE busy (and the
                HAM clock gate warm) through the tail. `anchor` (a tail
                intermediate) orders them into the right idle window."""
                if JUNK_OFF:
                    return
                for _ in range(k):
                    nc.tensor.matmul(junk_ps[:, :], anchor[:, 0:128],
                                     wh_sb[:, 0:256], start=True, stop=True,
                                     skip_group_check=True)

            gstate = {}

            def gemm_unit(r, n3, kw):
                """One x-GEMM matmul: row tile r, psum col block n3,
                contraction half kw. Returns a flush closure on kw==1."""
                xc = (xcA, xcB)[r // (NRT // 2)]
                rr = r % (NRT // 2)
                lhsT = xc[:, kw, 128 * rr:128 * rr + 128]
                if kw == 0:
                    gstate['px'] = pxpool.tile([128, 512], F32, tag="pX",
                                                name="pX")
                nc.tensor.matmul(gstate['px'][:, :], lhsT,
                                 wih[kw][:, 512 * n3:512 * (n3 + 1)],
                                 start=(kw == 0), stop=(kw == 1),
                                 skip_group_check=True)
                if kw == 1:
                    px = gstate['px']

                    def flush(r=r, n3=n3, px=px):
                        c0 = 1536 * r + 512 * n3
                        nc.scalar.copy(xgnat[:, c0:c0 + 256], px[:, 0:256])
                        nc.scalar.copy(xgnat[:, c0 + 256:c0 + 512],
                                       px[:, 256:512])
                    return flush
                return None

            def scatter(sp):
                """DMA xgnat row-tile -> packed ring buffer for step sp
                (step index within a chunk). xgnat group layout [r|z|n]
                lands on xp cols [r|z|xn] = 0:384."""
                r, tb = sp // NR, sp % NR
                for j in range(NG):
                    nc.sync.dma_start(
                        out=xp[sp % NR][32 * j:32 * j + BS, 0:GFD],
                        in_=xgnat[16 * tb:16 * tb + 16,
                                  1536 * r + 384 * j:1536 * r + 384 * (j + 1)])

            def seed(s, A, Bp):
                """Seed psum for step s: x projections + all biases."""
                nc.tensor.matmul(A[:, :], imod, xp[s % NR][:, 0:GFD],
                                 start=True, stop=False,
                                 tile_position=(0, 0),
                                 skip_group_check=True)
                nc.tensor.matmul(Bp[:, :], imod, xp[s % NR][:, GFD:PB],
                                 start=True, stop=False,
                                 tile_position=(0, 0),
                                 skip_group_check=True)

            # GEMM unit schedule: step -> list of (r, n3, kw)
            gsched = {}
            for r in range(NRT):
                # base > 8r: row r's flushes must trail this chunk's
                # scatters of row r (which end at s=8r+1); base 8r+2 also
                # keeps xcB rows (r >= NRT/2) after the s=HB-1 xcB refill
                base = 8 * r + 2
                units = [(n3, kw) for n3 in range(3) for kw in range(2)]
                for i, (n3, kw) in enumerate(units):
                    gsched.setdefault(base + i, []).append((r, n3, kw))

            def step(s, A, Bp, ABnext):
                """Emit one timestep (recurrent waves + tail)."""
                p = s % 2

                # --- A-waves (r|z, N=256) then B-waves (hn, N=128) ---
                for c in range(4):
                    for j in range(NG):
                        oo = slice(32 * j, 32 * (j + 1))
                        nc.tensor.matmul(
                            A[oo, 0:2 * HC],
                            hT_sb[1 - p][:, 32 * c:32 * (c + 1)],
                            whh_k[c][:, j * GFD:j * GFD + 2 * HC],
                            start=False, stop=(c == 3 and j == NG - 1),
                            tile_position=(0, 32 * j),
                            skip_group_check=True)
                for c in range(4):
                    for j in range(NG):
                        oo = slice(32 * j, 32 * (j + 1))
                        nc.tensor.matmul(
                            Bp[oo, 0:HC],
                            hT_sb[1 - p][:, 32 * c:32 * (c + 1)],
                            whh_k[c][:, j * GFD + 2 * HC:(j + 1) * GFD],
                            start=False, stop=(c == 3 and j == NG - 1),
                            tile_position=(0, 32 * j),
                            skip_group_check=True)

                # --- seed next step's psum (runs during this tail) ---
                if ABnext is not None:
                    seed(s + 1, *ABnext)

                # --- x-GEMM units assigned to this step slot ---
                flushes = []
                for unit in gsched.get(s, ()):
                    f = gemm_unit(*unit)
                    if f is not None:
                        flushes.append(f)

                # --- elementwise tail (fp16) ---
                r_t = wpool.tile([128, HC], F16, tag="r")
                z_t = wpool.tile([128, HC], F16, tag="z")
                m = wpool.tile([128, HC], F16, tag="m")
                a = wpool.tile([128, HC], F16, tag="a")
                ndt = F16 if SINGLE_T else F32
                n_t = wpool.tile([128, HC], ndt, tag="n")
                u = wpool.tile([128, HC], F16, tag="u")
                v = wpool.tile([128, HC], ndt, tag="v")
                pT = ptpool.tile([128, 128], ndt, tag="pT")

                junkmm(1, hT_sb[1 - p])
                nc.scalar.activation(r_t[:], A[:, 0:HC], sig)
                nc.scalar.activation(z_t[:], A[:, HC:2 * HC], sig)
                nc.vector.tensor_tensor(m[:], r_t[:], Bp[:, 0:HC],
                                        mybir.AluOpType.mult)
                junkmm(2, r_t)
                nc.vector.tensor_tensor(a[:], m[:], A[:, 2 * HC:3 * HC],
                                        mybir.AluOpType.add)
                nc.scalar.activation(n_t[:], a[:], tanh)
                nc.vector.tensor_tensor(u[:], hprev[1 - p][:, :], n_t[:],
                                        mybir.AluOpType.subtract)
                if not SINGLE_T:
                    # T(n) accumulates into pT (fp32); T(v) completes h'^T
                    nc.tensor.matmul(pT[:, :], n_t[:, :], id_f32,
                                     is_transpose=True, start=True, stop=False,
                                     skip_group_check=True)
                nc.vector.tensor_tensor(v[:], z_t[:], u[:],
                                        mybir.AluOpType.mult)
                junkmm(2, u)
                if not SINGLE_T:
                    nc.tensor.matmul(pT[:, :], v[:, :], id_f32,
                                     is_transpose=True, start=False, stop=True,
                                     skip_group_check=True)
                nc.vector.tensor_tensor(hprev[p][:, :], n_t[:], v[:],
                                        mybir.AluOpType.add)
                if SINGLE_T:
                    nc.tensor.matmul(pT[:, :], hprev[p][:, :], id_f16,
                                     is_transpose=True, start=True, stop=True,
                                     skip_group_check=True)
                junkmm(1, hprev[p])
                nc.vector.tensor_copy(hT_sb[p][:, :], pT[:, :])

                for f in flushes:
                    f()

            # ================= prologue: chunk 0 =================
            dma_x(xcA, 0)
            dma_x(xcB, HB * BS)
            for r in range(NRT):
                for n3 in range(3):
                    fl = None
                    for kw in range(2):
                        f = gemm_unit(r, n3, kw)
                        fl = f or fl
                    fl()
            for sp in range(SCD):
                scatter(sp)

            # ================= main loop =================
            with tc.For_i(0, T * BS, CH * BS,
                          hint_engines=tuple(mybir.ALL_ENGINES)) as iv:
                dma_x(xcA, iv + CH * BS)       # next chunk, first half
                A = papool.tile([128, GFD], F32, tag="pA")
                Bp = pbpool.tile([128, HC], F32, tag="pB")
                seed(0, A, Bp)
                for s in range(CH):
                    if s + 1 < CH:
                        ABnext = (papool.tile([128, GFD], F32, tag="pA",
                                              name="pA"),
                                  pbpool.tile([128, HC], F32, tag="pB",
                                              name="pB"))
                    else:
                        ABnext = None
                    step(s, A, Bp, ABnext)
                    if ABnext is not None:
                        A, Bp = ABnext
                    # scatter for step s+SCD (wraps into next chunk)
                    scatter((s + SCD) % CH)
                    if s == HB - 1:
                        dma_x(xcB, iv + CH * BS + HB * BS)

            # final h lives in hprev[(T-1) % 2]
            nc.sync.dma_start(out=hout[:, :], in_=hprev[(T - 1) % 2][0:112, :])

    _split_sync_waits(nc)
    return nc


def _split_sync_waits(nc):
    """Walrus codegen allows exactly ONE sync wait per instruction (the TPB
    events struct has a single wait slot). Tile emits multi-wait
    instructions (loop back-edge drains, barrier NoOps, cross-engine RAW
    joins); split the extras onto same-engine NoOps inserted immediately
    before -- the sequencer processes them in order, so semantics are
    identical."""
    for blk in nc.m.functions[0].blocks:
        i = 0
        while i < len(blk.instructions):
            inst = blk.instructions[i]
            si = getattr(inst, "sync_info", None)
            if si and si.on_wait and len(si.on_wait) > 1:
                waits = list(si.on_wait)
                si.on_wait = [waits[-1]]
                for w in waits[:-1]:
                    nop = mybir.InstNoOp(
                        name=nc.get_next_instruction_name(), ins=[], outs=[])
                    nop.engine = inst.engine
                    nop.sync_info = mybir.SyncInfo(on_wait=[w], on_update=[])
                    nc.register_instruction(nop)
                    blk.instructions.insert(i, nop)
                    i += 1
            i += 1


_NC_CACHE = {}


def run(x, W_ih, W_hh, b_ih, b_hh, trace=False):
    from concourse.bass_utils import run_bass_kernel_spmd

    x = np.asarray(x, dtype=np.float32)
    W_ih = np.asarray(W_ih, dtype=np.float32)
    W_hh = np.asarray(W_hh, dtype=np.float32)
    b_ih = np.asarray(b_ih, dtype=np.float32)
    b_hh = np.asarray(b_hh, dtype=np.float32)

    key = (x.shape[1],)
    if key not in _NC_CACHE:
        _NC_CACHE[key] = build_kernel(T=x.shape[1])
    nc = _NC_CACHE[key]

    wts = host_prepare_weights(W_ih, W_hh, b_ih, b_hh)
    ident32 = np.ascontiguousarray(np.eye(128, dtype=np.float32))
    in_maps = [{"xpack": host_blob(x, wts["wpack"], c), "whhb": wts["whhb"],
                "auxw": ident32}
               for c in range(NCORES)]
    res = run_bass_kernel_spmd(nc, in_maps, list(range(NCORES)), trace=trace)
    h = np.zeros((B, H), np.float32)
    for c in range(NCORES):
        h[c * BS:(c + 1) * BS] = host_post(np.asarray(res.results[c]["hout"]))
    return h, res


def kernel(x, W_ih, W_hh, b_ih, b_hh):
    h, _ = run(x, W_ih, W_hh, b_ih, b_hh)
    return h
